# revision 1
# baseline (speedup 1.0000x reference)
"""EMA (first-order IIR) forward kernel for Trainium2, SPMD over 8 NeuronCores.

y[b, c, t] = gamma[c] * y[b, c, t-1] + (1 - gamma[c]) * x[b, c, t],  y[.., -1] = 0
gamma = sigmoid(weight)

Sharding: data-parallel over B (8 batches -> 8 cores, zero communication).
Per core: x_shard [C=512, T=8192] fp32. Channels go on SBUF partitions
(4 groups of 128); the recurrence along T runs on the DVE via
tensor_tensor_scan (state = gamma*state + x) in chunks of F columns,
carry-chained through each chunk's last column.

Pipeline per chunk (post-scale formulation, y = (1-gamma) * scan(gamma, x)):
  sync ring   : DMA-in x chunk                     (qSP HWDGE, streams freely)
  DVE         : tensor_tensor_scan (only DVE work; gamma is a stride-0
                broadcast AP over a [128,1] column, nothing materialized)
  ACT         : scale by (1-gamma) per partition
  ACT ring    : DMA-out right after the scale in same-engine program order,
                so the out ring never head-of-line blocks on a semaphore
"""

import os

import numpy as np

import concourse.bass as bass
import concourse.tile as tile
from concourse import bacc, mybir
from concourse.bass_utils import run_bass_kernel_spmd

B, C, T = 8, 512, 8192
P = 128              # SBUF partition count
NG = C // P          # channel groups per core
F = int(os.environ.get("EMA_F", "4096"))   # max scan chunk (free-dim columns)
# Per-group chunk schedule along T. A small first chunk lets the first scan
# start as soon as ~0.5 MiB has landed instead of waiting for a full 2 MiB.
_sched = os.environ.get("EMA_SCHED", "1024,3072,3072,1024")
CHUNKS = [int(c) for c in _sched.split(",")] if _sched else [F] * (T // F)
assert sum(CHUNKS) == T, CHUNKS
N_CORES = 8

# gamma operand for the scan: stride-0 broadcast AP (default) or a
# materialized [P, F] tile (fallback if HW rejects stride-0 reads).
BCAST_AP = os.environ.get("EMA_BCAST_AP", "1") == "1"
XBUFS = int(os.environ.get("EMA_XBUFS", "5"))
YSBUFS = int(os.environ.get("EMA_YSBUFS", "6"))
YOBUFS = int(os.environ.get("EMA_YOBUFS", "3"))

LAST_RESULT = None   # BassKernelResults of the most recent run (for test.py)

_prog_cache = {}


def _build_program():
    key = (tuple(CHUNKS), BCAST_AP, XBUFS, YSBUFS, YOBUFS)
    if key in _prog_cache:
        return _prog_cache[key]

    nc = bacc.Bacc("TRN2", target_bir_lowering=False, debug=False)
    f32 = mybir.dt.float32

    x_d = nc.dram_tensor("x", [C, T], f32, kind="ExternalInput").ap()
    g_d = nc.dram_tensor("g", [C, 1], f32, kind="ExternalInput").ap()
    og_d = nc.dram_tensor("og", [C, 1], f32, kind="ExternalInput").ap()
    y_d = nc.dram_tensor("y", [C, T], f32, kind="ExternalOutput").ap()

    xv = x_d.rearrange("(g p) t -> g p t", p=P)
    yv = y_d.rearrange("(g p) t -> g p t", p=P)
    gv = g_d.rearrange("(g p) o -> g p o", p=P)
    ogv = og_d.rearrange("(g p) o -> g p o", p=P)

    with tile.TileContext(nc) as tc:
        with (
            tc.tile_pool(name="cols", bufs=1) as cols,
            tc.tile_pool(name="gb", bufs=2) as gbp,
            tc.tile_pool(name="xin", bufs=XBUFS) as xp,
            tc.tile_pool(name="ys", bufs=YSBUFS) as ysp,
            tc.tile_pool(name="yo", bufs=YOBUFS) as yop,
        ):
            # gamma / (1-gamma) columns, hoisted and issued on the ACT ring so
            # the sync ring's head is the first x chunk. Each [128,1] column is
            # 512 contiguous bytes in DRAM -> a single-descriptor DMA.
            g_cols, og_cols = [], []
            for gi in range(NG):
                g_sb = cols.tile([P, 1], f32, tag=f"gcol{gi}")
                nc.scalar.dma_start(g_sb[:], gv[gi])
                g_cols.append(g_sb)
                og_sb = cols.tile([P, 1], f32, tag=f"ogcol{gi}")
                nc.scalar.dma_start(og_sb[:], ogv[gi])
                og_cols.append(og_sb)

            # Interleave groups chunk-by-chunk: all four small first chunks
            # land early, so the DVE ramp has four independent scans to run
            # back-to-back instead of idling until the first 3072-wide chunk
            # arrives. Carries stay per-group.
            prev = [None] * NG
            prev_w = [0] * NG
            t0 = 0
            for fk in CHUNKS:
                for gi in range(NG):
                    g_sb = g_cols[gi][:]
                    og_sb = og_cols[gi][:]
                    xt = xp.tile([P, fk], f32, tag="x")
                    nc.sync.dma_start(xt[:], xv[gi, :, t0:t0 + fk])
                    ys = ysp.tile([P, fk], f32, tag="ys")
                    init = (0.0 if prev[gi] is None
                            else prev[gi][:, prev_w[gi] - 1:prev_w[gi]])
                    nc.vector.tensor_tensor_scan(
                        ys[:], g_sb.broadcast_to([P, fk]), xt[:], init,
                        mybir.AluOpType.mult, mybir.AluOpType.add,
                    )
                    yo = yop.tile([P, fk], f32, tag="yo")
                    nc.scalar.activation(
                        yo[:], ys[:], mybir.ActivationFunctionType.Copy,
                        scale=og_sb,
                    )
                    nc.scalar.dma_start(yv[gi, :, t0:t0 + fk], yo[:])
                    prev[gi] = ys
                    prev_w[gi] = fk
                t0 += fk

    nc.compile()
    _prog_cache[key] = nc
    return nc


def kernel(x: np.ndarray, weight: np.ndarray) -> np.ndarray:
    global LAST_RESULT
    assert x.shape == (B, C, T) and weight.shape == (C,)

    x = np.ascontiguousarray(x, dtype=np.float32)
    gamma = (1.0 / (1.0 + np.exp(-weight.astype(np.float64)))).astype(np.float32)
    one_minus_gamma = (np.float32(1.0) - gamma).astype(np.float32)
    g_in = gamma.reshape(C, 1)
    og_in = one_minus_gamma.reshape(C, 1)

    nc = _build_program()
    in_maps = [{"x": x[i], "g": g_in, "og": og_in} for i in range(N_CORES)]
    trace = os.environ.get("EMA_TRACE", "0") == "1"
    LAST_RESULT = run_bass_kernel_spmd(
        nc, in_maps, list(range(N_CORES)), trace=trace,
    )
    out = np.stack([LAST_RESULT.results[i]["y"] for i in range(N_CORES)])
    return out.astype(np.float32, copy=False)

